# revision 10
# baseline (speedup 1.0000x reference)
"""Trainium2 Bass kernel for nn_EquivariantConvolutionBlock (sparse 5^3 equivariant
conv + gate + batchnorm over 300k voxels in a 128^3 grid), SPMD over 8 NeuronCores.

Strategy (per core = one x-slab of 16 grid planes, halo +-2):
- Host folds the e3nn tensor-product kernel + self-connection into per-window
  stationary matrices [128,112] (4 stencil z-offsets x 32 ch contracted per matmul).
- Host builds a fp16 "neighborhood block" volume B: each 1KB row = a [4dy x 4dz]
  block of 32-ch features (sliding windows in both y and z, yq-major row order so
  consecutive gather descriptors land on nearby HBM rows).
- 8 window-blocks per voxel cover the whole active 5^3 stencil (81 offsets incl.
  the folded self-connection). One dma_gather per (plane, window-block) with a
  runtime voxel count (trailing -1 idx are skipped by SWDGE) builds the rhs;
  30 nonzero block-column matmuls per 512-voxel tile accumulate in PSUM;
  sigmoid/gate on ACT/DVE; the gated pre-BN s|v streams out as fp16.
- BatchNorm batch statistics + normalization are applied on the host (exact
  two-pass fp32 stats over the fp16 device output; avoids the device AllReduce
  and a second device pass entirely).
"""
import sys

sys.path.insert(0, "/opt/trn_rl_repo")

import os
import numpy as np
from contextlib import ExitStack

import concourse.bass as bass
import concourse.bacc as bacc
import concourse.tile as tile
import concourse.mybir as mybir
from concourse.bass_utils import run_bass_kernel_spmd

F16 = mybir.dt.float16
F32 = mybir.dt.float32
I16 = mybir.dt.int16
I32 = mybir.dt.int32

N = 300000
GRID = 128
NCORES = 8
EPS = 1e-5
PPC = 16          # planes per core
YQ = 132          # y-block-start axis (outer, row-major)
SD = 132          # z-window-start axis (inner)
PLANE_ROWS = YQ * SD          # 17424
BROWS = 20 * PLANE_ROWS       # B-volume rows per core (20 x-planes incl halo)
TCOLS = 512
PAD_IDX = 130     # row (yq=0, s=130): all-zero elem

# window-blocks: (dx, ady, adz); block covers dy in [ady, ady+3], dz in [adz, adz+3]
WBS = [(-2, -1, -1),
       (-1, -2, -2), (-1, -1, -1),
       (0, -2, -2), (0, -1, -1),
       (1, -2, -2), (1, -1, -1),
       (2, -1, -1)]

_COMPILED = None


# ---------------------------------------------------------------- host math

def _soft_unit_step(t):
    out = np.zeros_like(t)
    m = t > 0
    out[m] = np.exp(-1.0 / t[m])
    return out


def _make_ker_by_off(tp_weight, Ws1, Ws2, Wv):
    ax = np.arange(-2, 3.0)
    lat = np.stack(np.meshgrid(ax, ax, ax, indexing="ij"), -1).reshape(-1, 3)
    d = np.linalg.norm(lat, axis=-1)
    values = np.linspace(0.0, 2.5, 5)[1:-1]
    step = 2.5 / 4
    diff = (d[..., None] - values) / step
    emb = 1.14136 * float(np.e ** 2) * _soft_unit_step(diff + 1.0) * _soft_unit_step(1.0 - diff)
    w = (emb @ tp_weight.astype(np.float64)) / 125.0
    w1, w2, w3, w4, w5, w6 = [w[:, i * 128:(i + 1) * 128].reshape(-1, 8, 16) for i in range(6)]
    unit = np.where(d[:, None] > 0, lat / np.where(d > 0, d, 1.0)[:, None], 0.0)
    y1 = np.sqrt(3.0) * unit
    A = 0.25
    B = A / np.sqrt(3.0)
    Cc = 0.25
    M_ss = A * w1
    M_vs = (B * np.einsum("xi,xuw->xuiw", y1, w2)).reshape(-1, 24, 16)
    M_sg = A * w3
    M_vg = (B * np.einsum("xi,xuw->xuiw", y1, w4)).reshape(-1, 24, 16)
    M_sv = (Cc * np.einsum("xk,xuw->xuwk", y1, w5)).reshape(-1, 8, 48)
    M_vv = (Cc * np.einsum("xuw,ik->xuiwk", w6, np.eye(3))).reshape(-1, 24, 48)
    top = np.concatenate([M_ss, M_sg, M_sv], -1)
    bot = np.concatenate([M_vs, M_vg, M_vv], -1)
    ker = np.concatenate([top, bot], 1)            # [125,32,80]
    kbo = {tuple(int(v) for v in lat[i]): ker[i] for i in range(125)}
    inv = 1.0 / np.sqrt(8.0)
    Wsc = np.zeros((32, 80))
    Wsc[0:8, 0:16] = Ws1 * inv
    Wsc[0:8, 16:32] = Ws2 * inv
    u, w_ = np.meshgrid(np.arange(8), np.arange(16), indexing="ij")
    for i in range(3):
        Wsc[8 + u * 3 + i, 32 + w_ * 3 + i] = Wv * inv
    kbo[(0, 0, 0)] = kbo[(0, 0, 0)] + Wsc          # emb(0)=0, so center slot is free
    return kbo


def _active(dx, dy, dz):
    d2 = dx * dx + dy * dy + dz * dz
    return (0 < d2 <= 6) or (dx, dy, dz) == (0, 0, 0)


def _assigned_wb(dx, dy, dz):
    """Index into WBS owning cell (dx,dy,dz); each active cell owned once."""
    for i, (wdx, ady, adz) in enumerate(WBS):
        if wdx == dx and ady <= dy <= ady + 3 and adz <= dz <= adz + 3:
            return i
    raise AssertionError((dx, dy, dz))


def _nonzero_slots():
    nz = []
    for wbi, (dx, ady, adz) in enumerate(WBS):
        for q in range(4):
            dy = ady + q
            for k in range(4):
                dz = adz + k
                if (abs(dy) <= 2 and abs(dz) <= 2 and _active(dx, dy, dz)
                        and _assigned_wb(dx, dy, dz) == wbi):
                    nz.append((wbi, q))
                    break
    return nz


def _build_stationaries(kbo):
    """[128, 32*112] f16: slot (wbi*4+q) = column (dx, ady+q), rows 32k+c = dz=adz+k.
    Output channel layout: 0:16 s | 32:48 gates | 64:112 v."""
    kers = np.zeros((128, len(WBS) * 4 * 112), np.float32)
    for wbi, (dx, ady, adz) in enumerate(WBS):
        for q in range(4):
            dy = ady + q
            c0 = (wbi * 4 + q) * 112
            for k in range(4):
                dz = adz + k
                if abs(dy) > 2 or abs(dz) > 2 or not _active(dx, dy, dz):
                    continue
                if _assigned_wb(dx, dy, dz) != wbi:
                    continue
                m = kbo[(dx, dy, dz)]
                kers[32 * k:32 * (k + 1), c0 + 0:c0 + 16] = m[:, 0:16]
                kers[32 * k:32 * (k + 1), c0 + 32:c0 + 48] = m[:, 16:32]
                kers[32 * k:32 * (k + 1), c0 + 64:c0 + 112] = m[:, 32:80]
    return kers.astype(np.float16)


def _wrap_idx(flat):
    w16 = flat.reshape(-1, 16).T.astype(np.int16)
    return np.tile(w16, (8, 1))


# ---------------------------------------------------------------- device program

def _build_program(TPP):
    PCOLS = TPP * TCOLS
    nzset = set(_nonzero_slots())
    IDXW = PPC * len(WBS) * PCOLS // 16

    nc = bacc.Bacc("TRN2", target_bir_lowering=False, debug=False,
                   num_devices=NCORES, num_swdge_queues=4)
    B_t = nc.dram_tensor("bvol", [BROWS, 512], F16, kind="ExternalInput").ap()
    IDX_t = nc.dram_tensor("idx", [128, IDXW], I16, kind="ExternalInput").ap()
    KER_t = nc.dram_tensor("kers", [128, len(WBS) * 4 * 112], F16,
                           kind="ExternalInput").ap()
    E_t = nc.dram_tensor("emat", [16, 48], F32, kind="ExternalInput").ap()
    OUT_t = nc.dram_tensor("out", [64, PPC * PCOLS], F16, kind="ExternalOutput").ap()

    gather_ct = 0

    with tile.TileContext(nc) as tc, ExitStack() as ctx:
        cpool = ctx.enter_context(tc.tile_pool(name="const", bufs=1))
        rpool = ctx.enter_context(tc.tile_pool(name="rhs", bufs=12))
        spool = ctx.enter_context(tc.tile_pool(name="small", bufs=3))
        svpool = ctx.enter_context(tc.tile_pool(name="svp", bufs=2))
        pp = ctx.enter_context(tc.tile_pool(name="psA", bufs=6, space="PSUM"))
        pg = ctx.enter_context(tc.tile_pool(name="psB", bufs=2, space="PSUM"))

        kers = cpool.tile([128, len(WBS) * 4 * 112], F16)
        nc.sync.dma_start(kers[:], KER_t[:])
        e48 = cpool.tile([48, 48], F32)
        nc.sync.dma_start(e48[32:48, :], E_t[:])
        idx_sb = cpool.tile([128, IDXW], I16)
        nc.sync.dma_start(idx_sb[:], IDX_t[:])

        nz_by_wb = {}
        for wbi, q in sorted(nzset):
            nz_by_wb.setdefault(wbi, []).append(q)
        wb_order = sorted(nz_by_wb.keys())
        first_wb, last_wb = wb_order[0], wb_order[-1]

        for j in range(PPC):
            ps = [pp.tile([112, TCOLS], F32, tag="convps", name=f"ps_{j}_{k}")
                  for k in range(TPP)]
            for wbi in wb_order:
                dx, ady, adz = WBS[wbi]
                base = (j + 2 + dx) * PLANE_ROWS
                rhss = []
                for sub in range(TPP):
                    off16 = ((j * len(WBS) + wbi) * PCOLS + sub * TCOLS) // 16
                    rhs = rpool.tile([128, 4, TCOLS], F16, tag="rhs",
                                     name=f"rh_{j}_{wbi}_{sub}")
                    nc.gpsimd.dma_gather(
                        rhs[:], B_t[base: base + PLANE_ROWS, :],
                        idx_sb[:, off16: off16 + TCOLS // 16],
                        TCOLS, TCOLS, 512, transpose=True,
                        queue_num=gather_ct % 4,
                    )
                    gather_ct += 1
                    rhss.append(rhs)
                for q in nz_by_wb[wbi]:
                    for sub in range(TPP):
                        nc.tensor.matmul(
                            ps[sub][:],
                            kers[:, (wbi * 4 + q) * 112:(wbi * 4 + q + 1) * 112],
                            rhss[sub][:, q, :],
                            start=(wbi == first_wb and q == nz_by_wb[wbi][0]),
                            stop=(wbi == last_wb and q == nz_by_wb[wbi][-1]),
                        )
            # post-processing per 512-col tile -> gated s|v fp16
            svp = svpool.tile([112, PCOLS], F16, tag="svp", name=f"svp_{j}")
            for sub in range(TPP):
                c0 = sub * TCOLS
                sig = spool.tile([48, TCOLS], F32, tag="sig")
                nc.scalar.activation(sig[:], ps[sub][0:48, :],
                                     mybir.ActivationFunctionType.Sigmoid)
                gex = pg.tile([112, TCOLS], F32, tag="gexps")
                nc.tensor.matmul(gex[64:112, :], e48[32:48, :], sig[32:48, :],
                                 start=True, stop=True, tile_position=(32, 64))
                nc.vector.tensor_tensor(svp[0:16, c0:c0 + TCOLS],
                                        ps[sub][0:16, :], sig[0:16, :],
                                        mybir.AluOpType.mult)
                gexs = spool.tile([112, TCOLS], F32, tag="gexs")
                nc.scalar.activation(gexs[64:112, :], gex[64:112, :],
                                     mybir.ActivationFunctionType.Copy)
                nc.vector.tensor_tensor(svp[64:112, c0:c0 + TCOLS],
                                        ps[sub][64:112, :], gexs[64:112, :],
                                        mybir.AluOpType.mult)
            nc.sync.dma_start(OUT_t[0:16, j * PCOLS:(j + 1) * PCOLS], svp[0:16, :])
            nc.sync.dma_start(OUT_t[16:64, j * PCOLS:(j + 1) * PCOLS],
                              svp[64:112, :])

    nc.compile()
    return nc


# ---------------------------------------------------------------- host driver

def _prep_inputs(inputs, TPP):
    PCOLS = TPP * TCOLS
    feats = np.asarray(inputs["feats"], np.float32)
    coords = np.asarray(inputs["coords"], np.int64)
    kbo = _make_ker_by_off(np.asarray(inputs["tp_weight"], np.float64),
                           np.asarray(inputs["Ws1"], np.float64),
                           np.asarray(inputs["Ws2"], np.float64),
                           np.asarray(inputs["Wv"], np.float64))
    kers = _build_stationaries(kbo)
    E = np.zeros((16, 48), np.float32)
    for w in range(16):
        for i in range(3):
            E[w, w * 3 + i] = 1.0

    lin = (coords[:, 0] * GRID + coords[:, 1]) * GRID + coords[:, 2]
    perm = np.argsort(lin, kind="stable")
    cs = coords[perm]
    fs = feats[perm].astype(np.float16)

    from numpy.lib.stride_tricks import sliding_window_view
    in_maps = []
    counts = np.zeros((NCORES, PPC), np.int64)
    for c in range(NCORES):
        x0 = 16 * c
        m = (cs[:, 0] >= x0 - 2) & (cs[:, 0] < x0 + 18)
        cc, ff = cs[m], fs[m]
        # V slots: [20 xp, 136 yp(pad), 136 zp] of 32 fp16
        Vs = np.zeros((20, 136, 136, 32), np.float16)
        Vs[cc[:, 0] - x0 + 2, cc[:, 1] + 2, cc[:, 2] + 2] = ff
        # A4[xp, yp, s] = Vs[xp, yp, s:s+4] as [4dz,32ch] -> [20,136,132,128]
        A4 = sliding_window_view(Vs, 4, axis=2)[:, :, :SD]     # [20,136,132,32,4]
        A4 = np.ascontiguousarray(A4.transpose(0, 1, 2, 4, 3)).reshape(20, 136, SD, 128)
        # B[xp, yq, s] = concat_k A4[xp, yq+k, s]  (yq-major row order)
        sw = sliding_window_view(A4, 4, axis=1)                # [20,133,132,128,4]
        Bv = np.ascontiguousarray(
            sw[:, :YQ].transpose(0, 1, 2, 4, 3)).reshape(BROWS, 512)

        mloc = (cs[:, 0] >= x0) & (cs[:, 0] < x0 + 16)
        cl = cs[mloc]
        idx_blocks = []
        for j in range(PPC):
            pm = cl[:, 0] == x0 + j
            y, z = cl[pm, 1], cl[pm, 2]
            n = len(y)
            assert 0 < n <= PCOLS, f"plane count {n} not in (0, {PCOLS}]"
            counts[c, j] = n
            for (dx, ady, adz) in WBS:
                blk = np.full(PCOLS, PAD_IDX, np.int64)
                blk[:n] = (y + 2 + ady) * SD + (z + 2 + adz)
                idx_blocks.append(blk)
        idx = _wrap_idx(np.concatenate(idx_blocks))
        in_maps.append({"bvol": Bv, "idx": idx, "kers": kers, "emat": E})
    return in_maps, counts, perm, cs


def _axon_reset():
    """Recover a wedged NeuronCore (NRT_EXEC_UNIT_UNRECOVERABLE) via the axon
    PJRT plugin's reset entry point. Best-effort."""
    try:
        import ctypes
        lib = ctypes.CDLL("/opt/axon/libaxon_pjrt.so")
        lib.axon_reset.restype = ctypes.c_int64
        return int(lib.axon_reset())
    except Exception:
        return -1


def kernel(**inputs):
    global _COMPILED
    coords = np.asarray(inputs["coords"], np.int64)
    maxp = int(np.bincount(coords[:, 0], minlength=GRID).max())
    TPP = max(5, -(-maxp // TCOLS))
    if _COMPILED is None or _COMPILED[0] != TPP:
        nc = _build_program(TPP)
        _COMPILED = (TPP, nc)
    else:
        nc = _COMPILED[1]
    PCOLS = TPP * TCOLS
    in_maps, counts, perm, cs = _prep_inputs(inputs, TPP)
    sv = None
    for attempt in range(3):
        try:
            res = run_bass_kernel_spmd(nc, in_maps, core_ids=list(range(NCORES)))
            pieces = []
            for c in range(NCORES):
                o = res.results[c]["out"]
                for j in range(PPC):
                    n = counts[c, j]
                    if n:
                        pieces.append(o[:, j * PCOLS: j * PCOLS + n])
            sv = np.concatenate(pieces, axis=1).T.astype(np.float32)  # [N,64]
            if np.isfinite(sv).all():
                break
        except Exception:
            if attempt == 2:
                raise
        _axon_reset()
    # ---- host BatchNorm (exact two-pass stats, as in the reference)
    bn_w = np.asarray(inputs["bn_weight"], np.float32)
    bn_b = np.asarray(inputs["bn_bias"], np.float32)
    s, v = sv[:, :16], sv[:, 16:]
    mu = s.mean(0, dtype=np.float64).astype(np.float32)
    scn = s - mu
    var = (scn * scn).mean(0, dtype=np.float64).astype(np.float32)
    out_s = scn * ((var + EPS) ** -0.5 * bn_w[:16]) + bn_b
    v3 = v.reshape(-1, 16, 3)
    vn = np.einsum("ngi,ngi->g", v3, v3, dtype=np.float64) / (3.0 * len(v3))
    out_v = v3 * ((vn.astype(np.float32) + EPS) ** -0.5 * bn_w[16:])[None, :, None]
    sorted_out = np.concatenate([out_s, out_v.reshape(-1, 48)], -1)
    out = np.empty_like(sorted_out)
    out[perm] = sorted_out
    return out
